# revision 50
# baseline (speedup 1.0000x reference)
"""AGLISTA (adaptive-gain LISTA with top-k masking) Trainium2 kernel — v6.

Data-parallel over batch on 8 NeuronCores: B=2048 -> 256 samples/core as
2 software-pipelined groups of 128 samples (128 SBUF partitions). State x
kept as (128, N=2048) f32 per group. HW exec ~1.454ms (v2.1 baseline
1.954ms), rel err 5.5e-4 (gate 2e-2). Rank-select is emitted separately
from the shrink (rank before the other group's mmB) so it fills the
DVE stall where u-stt waits on mm2's PSUM banks; deferring the Rt
chain past the other shrink regresses (mm2 start gates on it). Latest gains: chunked shrink
tail + au so ACT/DVE pipeline across 1024-col chunks; mm2 pass-3
n-outer so each PSUM bank's u slice starts early; full Rt chain on
DVE/GpSimd (an mm1-dependent op on ACT stalls the bisect behind it);
all bisect STEPS=5 (odd S; S=4 diverged on HW); PSUM pools pst=3/ps1=1
so the R-transpose does not WAR-wait a gx-transpose batch.

Key design (evolution from v2.1 measured on traces):
- Error-compensated 3-pass split matmuls: 1-pass fp32r u-noise (~1e-4)
  flips top-k boundaries -> 2e-2 rel err (flips do NOT wash out through
  the iteration dynamics; measured sensitivity sigma->rel:
  1.5e-4->5e-2, 1e-5->1.3e-2, 3e-6->6e-3). mm1 (phi) bf16 hi/lo pair,
  mm2 (W) f32r hi/lo pair (10-bit truncation): Ah@Bh + Ah@Bl + Al@Bh,
  all 512-wide moving / 1 cycle/row. Params pre-split in prep; dynamic
  sides (gxT, Rt) split on the fly (hi = cast, lo = exact - hi,
  self-compensating). mm1 is FLIPPED (gxT stationary, phi moving) for
  the wide moving dim; 4 PE transposes recover R.
- Warm-started all-ACT bisection for the per-sample top-p threshold
  (center = prev Tsc + calibrated mean drift MU, window +-DELTA, 5-7
  steps), Sign+accum counting, ties biased UP; masked-max8 rank-8
  correction + khi==p fallback. i<3 exact via max8/match_replace.
  NOTE: reducing STEPS by 1 (even S) diverged on HW — keep as is.
- Shrink reformulated: nq = (au <= Tsc)*q (one stt) replaces
  keep-mask + copy_predicated; d = (u - x) - nq with e = u - x on
  GpSimd during the bisection; xn = x + (1+r)*d, r from
  reciprocal_approx_fast((|d|+eps)/a).
- In-order engine queues make emission order ~ execution order: each
  group's topk is emitted before the other group's mmB; au=|u| (ACT)
  emitted at its consumer; |d| via DVE stt (an ACT op there stalls the
  next bisect behind a DVE dependency).
- tensor_scalar/tensor_copy run 2x (2x_2p) in f32 when all-SBUF;
  scalar_tensor_tensor/tensor_tensor do not — op choice matters.
- PSUM: ps1 pr(1) + ps2 pc0..3(1 each) + pst pt(3) = 8 banks exactly
  (pst=3/ps1=1 beat 2/2 by ~40us; splitting mm2 into two 2-bank
  n-halves to free banks for pst=4/ps1=2 regressed ~30us).
All top-k decisions stay fp32 (fp16 decisions fail at 5e-2). ACT uses
only exp_and_others-table functions — no 1283ns table reloads.
"""

import numpy as np

M, N, K, B = 512, 2048, 16, 2048
NCORES = 8
BL = B // NCORES          # 256 samples per core
G = 2                     # sample groups of 128 per core
EPS = 0.01
P_SCHED = tuple(min(8 * (i + 1), N) for i in range(K))
NCHUNK = 2                # elementwise chunking for ACT gain ops
CH = N // NCHUNK

# warm-start bisection schedule (calibrated offline, seeds 0-3):
# center_i = Tsc_{i-1} + MU[i]; window = +-DELTA[i]; STEPS[i] probes.
MU = {3: -0.0167, 4: -0.0123, 5: -0.0091, 6: -0.0064, 7: -0.0040,
      8: -0.0018, 9: 0.0004, 10: 0.0024, 11: 0.0047, 12: 0.0069,
      13: 0.0089, 14: 0.0112, 15: 0.0136}
DELTA = {3: 0.0414, 4: 0.0266, 5: 0.0284, 6: 0.0257, 7: 0.0253,
         8: 0.0238, 9: 0.0218, 10: 0.0226, 11: 0.0316, 12: 0.0379,
         13: 0.0591, 14: 0.0592, 15: 0.1208}
STEPS = {3: 5, 4: 5, 5: 5, 6: 5, 7: 5, 8: 5, 9: 5, 10: 5, 11: 5,
         12: 5, 13: 5, 14: 5, 15: 5}
DIRECT_ITERS = 3          # i<3: exact max8/match_replace chains

_CACHE = {}


def _build(scal, n_iters=K):
    import math
    import concourse.bacc as bacc
    import concourse.mybir as mybir
    import concourse.tile as tile
    from concourse.masks import make_identity

    F32 = mybir.dt.float32
    F32R = mybir.dt.float32r
    BF16 = mybir.dt.bfloat16
    A = mybir.AluOpType
    AF = mybir.ActivationFunctionType
    AX = mybir.AxisListType

    gamma, theta, aa_, vv_, vu_, theta_init = scal

    nc = bacc.Bacc("TRN2", target_bir_lowering=False, debug=False,
                   num_devices=NCORES)

    phh_d = nc.declare_dram_parameter("phh", [128, 16, M], BF16,
                                      isOutput=False)
    phl_d = nc.declare_dram_parameter("phl", [128, 16, M], BF16,
                                      isOutput=False)
    wh_d = nc.declare_dram_parameter("wh", [128, 4, N], F32R,
                                     isOutput=False)
    wl_d = nc.declare_dram_parameter("wl", [128, 4, N], F32R,
                                     isOutput=False)
    yT_d = nc.declare_dram_parameter("yT", [128, 4, BL], F32, isOutput=False)
    out_d = nc.declare_dram_parameter("out", [BL, N], F32, isOutput=True)

    with tile.TileContext(nc) as tc:
        with (
            tc.tile_pool(name="pers", bufs=1) as pers,
            tc.tile_pool(name="ps1", bufs=1, space="PSUM") as ps1,
            tc.tile_pool(name="ps2", bufs=1, space="PSUM") as ps2,
            tc.tile_pool(name="pst", bufs=3, space="PSUM") as pst,
        ):
            def pt_(shape, dt_, nm):
                return pers.tile(shape, dt_, tag=nm, name=nm)

            # ---- persistent SBUF tensors ----
            phh = pt_([128, 16, M], BF16, "phh")
            phl = pt_([128, 16, M], BF16, "phl")
            wh = pt_([128, 4, N], F32R, "wh")
            wl = pt_([128, 4, N], F32R, "wl")
            yT = pt_([128, 4, BL], F32, "yT")
            # per-group state
            x = [pt_([128, N], F32, f"x{g}") for g in range(G)]
            u = [pt_([128, N], F32, f"u{g}") for g in range(G)]
            au = [pt_([128, N], F32, f"au{g}") for g in range(G)]
            q = [pt_([128, N], F32, f"q{g}") for g in range(G)]
            e = [pt_([128, N], F32, f"e{g}") for g in range(G)]
            # shared scratch: gain product in mmA, bisect Sign out / rank
            # mask / nq in topk (lifetimes follow emission order)
            gxb = pt_([128, N], F32, "gxb")
            gxTh = pt_([128, N], BF16, "gxTh")
            gxTl = pt_([128, N], BF16, "gxTl")
            RtTs = pt_([128, 512], F32, "RtTs")   # R^T sbuf copy
            Rt32 = pt_([128, 512], F32, "Rt32")   # R - yT exact
            RtH = pt_([128, 512], F32R, "RtH")
            RtL = pt_([128, 512], F32R, "RtL")
            ident = pt_([128, 128], F32, "ident")
            io8 = pt_([128, 8], F32, "io8")
            lnb = pt_([128, K], F32, "lnb")    # ln(tvu_i) Exp bias
            c2p = pt_([128, K], F32, "c2p")    # 2049-2p_i (dir Sign bias)
            # per-group top-k state ([128,1] f32)
            Tsc = [pt_([128, 1], F32, f"Tsc{g}") for g in range(G)]
            ptt = [[pt_([128, 1], F32, f"ptt{g}_{j}") for j in range(2)]
                   for g in range(G)]
            dirb = [pt_([128, 1], F32, f"dirb{g}") for g in range(G)]
            ssum = [pt_([128, 1], F32, f"ssum{g}") for g in range(G)]
            hi = [pt_([128, 1], F32, f"hi{g}") for g in range(G)]
            rr = [pt_([128, 1], F32, f"rr{g}") for g in range(G)]
            rr5 = [pt_([128, 1], F32, f"rr5{g}") for g in range(G)]
            m0 = [pt_([128, 1], F32, f"m0{g}") for g in range(G)]
            fb = [pt_([128, 1], F32, f"fb{g}") for g in range(G)]
            top8 = [pt_([128, 8], F32, f"top8{g}") for g in range(G)]
            t8 = [pt_([128, 8], F32, f"t8{g}") for g in range(G)]

            # ---- prologue ----
            nc.sync.dma_start(yT[:], yT_d[:])
            nc.sync.dma_start(wh[:], wh_d[:])
            nc.sync.dma_start(wl[:], wl_d[:])
            nc.sync.dma_start(phh[:], phh_d[:])
            nc.sync.dma_start(phl[:], phl_d[:])
            make_identity(nc, ident[:])
            for j in range(8):
                nc.vector.memset(io8[:, j:j + 1], float(j + 1))
            for g in range(G):
                nc.vector.memset(x[g][:], 0.0)
            for i_ in range(n_iters):
                tg_ = theta[i_] if i_ > 0 else theta_init
                nc.vector.memset(lnb[:, i_:i_ + 1],
                                 float(math.log(tg_ * vu_[i_])))
                if i_ >= DIRECT_ITERS:
                    nc.vector.memset(c2p[:, i_:i_ + 1],
                                     float(2049.0 - 2.0 * P_SCHED[i_]))

            def cs(t_, c):
                return t_[:, CH * c:CH * (c + 1)]

            def emit_mmA(g, i):
                """gain + transpose + 3-pass mm1 -> RtH/RtL for group g."""
                ysl = yT[:, :, 128 * g:128 * (g + 1)]
                if i == 0:
                    # R = -yT exactly, split into f32r pair
                    nc.vector.tensor_scalar_mul(RtH[:], ysl, -1.0)
                    nc.vector.scalar_tensor_tensor(
                        RtL[:], ysl, -1.0, RtH.bitcast(F32)[:],
                        A.mult, A.subtract)
                    return
                # gain: zP = tvu*exp(-v|x|) in u[g] (scratch);
                # gxb = (1+zP)*x
                for c in range(NCHUNK):
                    nc.scalar.activation(cs(u[g], c), cs(x[g], c), AF.Abs)
                    nc.scalar.activation(cs(u[g], c), cs(u[g], c), AF.Exp,
                                         scale=float(-vv_[i]),
                                         bias=lnb[:, i:i + 1])
                    nc.vector.scalar_tensor_tensor(
                        cs(gxb, c), cs(u[g], c), 1.0, cs(x[g], c),
                        A.add, A.mult)
                # 16 transposes of gxb (f32, exact), split to bf16 pair
                for b4 in range(4):
                    pt = pst.tile([128, 512], F32, tag="pt", name="pt")
                    for q_ in range(4):
                        k = b4 * 4 + q_
                        nc.tensor.transpose(
                            pt[:, 128 * q_:128 * (q_ + 1)],
                            gxb[:, 128 * k:128 * (k + 1)], ident[:])
                    sl = slice(512 * b4, 512 * (b4 + 1))
                    nc.scalar.activation(gxTh[:, sl], pt[:], AF.Copy)
                    nc.vector.tensor_tensor(gxTl[:, sl], pt[:],
                                            gxTh[:, sl], A.subtract)
                # 3-pass mm1 (bf16 pair): RT accumulated in one PSUM bank
                pr = ps1.tile([128, 512], F32, tag="pr", name="pr")
                for k in range(16):
                    lh = gxTh[:, 128 * k:128 * (k + 1)]
                    nc.tensor.matmul(pr[:], lh, phh[:, k, :],
                                     start=(k == 0), stop=False)
                    nc.tensor.matmul(pr[:], lh, phl[:, k, :],
                                     start=False, stop=False)
                for k in range(16):
                    nc.tensor.matmul(pr[:], gxTl[:, 128 * k:128 * (k + 1)],
                                     phh[:, k, :],
                                     start=False, stop=(k == 15))
                nc.vector.tensor_copy(RtTs[:], pr[:])
                # 4 transposes recover R (m_local, m_chunk*b)
                pq = pst.tile([128, 512], F32, tag="pt", name="ptr")
                for q_ in range(4):
                    nc.tensor.transpose(
                        pq[:, 128 * q_:128 * (q_ + 1)],
                        RtTs[:, 128 * q_:128 * (q_ + 1)], ident[:])
                nc.vector.tensor_tensor(Rt32[:], pq[:], ysl, A.subtract)
                nc.vector.tensor_copy(RtH[:], Rt32[:])
                nc.gpsimd.tensor_tensor(RtL[:], Rt32[:],
                                        RtH.bitcast(F32)[:], A.subtract)

            def emit_mmB(g, i):
                """3-pass mm2 (f32r pair) + u + au + q + e for group g."""
                ng_i = float(-gamma[i])
                th_i = float(theta[i])
                pcs = [ps2.tile([128, 512], F32, tag=f"pc{n}", name=f"pc{n}")
                       for n in range(4)]
                for k in range(4):
                    lh = RtH[:, 128 * k:128 * (k + 1)]
                    for n in range(4):
                        nc.tensor.matmul(pcs[n][:], lh,
                                         wh[:, k, 512 * n:512 * (n + 1)],
                                         start=(k == 0), stop=False)
                # passes 2+3 n-grouped: bank n completes after
                # 16 + 8*(n+1) matmuls (vs 36+) so its u slice, au and
                # the bisection start earlier
                for n in range(4):
                    for k in range(4):
                        lh = RtH[:, 128 * k:128 * (k + 1)]
                        nc.tensor.matmul(pcs[n][:], lh,
                                         wl[:, k, 512 * n:512 * (n + 1)],
                                         start=False, stop=False)
                    for k in range(4):
                        ll = RtL[:, 128 * k:128 * (k + 1)]
                        nc.tensor.matmul(pcs[n][:], ll,
                                         wh[:, k, 512 * n:512 * (n + 1)],
                                         start=False, stop=(k == 3))
                    nc.vector.scalar_tensor_tensor(
                        u[g][:, 512 * n:512 * (n + 1)], pcs[n][:], ng_i,
                        x[g][:, 512 * n:512 * (n + 1)], A.mult, A.add)
                # q = clamp(u, +-theta) (ts 2x);
                # e = u - x (GpSimd, hidden under the bisection)
                nc.vector.tensor_scalar(q[g][:], u[g][:], th_i, -th_i,
                                        A.min, A.max)
                nc.gpsimd.tensor_tensor(e[g][:], u[g][:], x[g][:],
                                        A.subtract)

            def emit_topk(g, i):
                """Tsc for group g; bisect is a self-contained ACT chain.
                au is computed here (right before its first consumer) so
                it does not block unrelated ops in the in-order ACT queue."""
                p = float(P_SCHED[i])
                for c in range(NCHUNK):
                    nc.scalar.activation(cs(au[g], c), cs(u[g], c), AF.Abs)
                if i < DIRECT_ITERS:
                    rounds = P_SCHED[i] // 8
                    src_ = au[g]
                    nc.vector.max(top8[g][:], src_[:])
                    for rnd in range(1, rounds):
                        scr = u[g] if rnd % 2 == 1 else gxb
                        nc.vector.match_replace(
                            out=scr[:], in_to_replace=top8[g][:],
                            in_values=src_[:], imm_value=-1.0)
                        src_ = scr
                        nc.vector.max(top8[g][:], src_[:])
                    nc.vector.tensor_copy(Tsc[g][:], top8[g][:, 7:8])
                    return
                d_ = DELTA[i]
                W0 = 2.0 * d_
                S = STEPS[i]
                nc.scalar.activation(ptt[g][0][:], Tsc[g][:], AF.Copy,
                                     scale=1.0, bias=float(MU[i]))
                for s in range(S):
                    w = W0 / float(2 ** (s + 2))
                    pa = ptt[g][s % 2]
                    pb = ptt[g][(s + 1) % 2]
                    # count via Sign accum: ssum = n_lt - n_gt
                    nc.scalar.activation(u[g][:], au[g][:], AF.Sign,
                                         scale=-1.0, bias=pa[:],
                                         accum_out=ssum[g][:])
                    # dir = Sign((2049-2p) - ssum)  (ties move UP)
                    nc.scalar.activation(dirb[g][:], ssum[g][:], AF.Sign,
                                         scale=-1.0, bias=c2p[:, i:i + 1])
                    # ptt' = w*dir + ptt (Identity allows AP bias)
                    nc.scalar.activation(pb[:], dirb[g][:], AF.Identity,
                                         scale=float(w), bias=pa[:])
                w_last = W0 / float(2 ** (S + 1))
                pfin = ptt[g][S % 2]
                nc.scalar.activation(hi[g][:], pfin[:], AF.Copy,
                                     scale=1.0, bias=float(w_last))
                # khi count at hi
                nc.scalar.activation(u[g][:], au[g][:], AF.Sign,
                                     scale=-1.0, bias=hi[g][:],
                                     accum_out=ssum[g][:])

            def emit_rank(g, i):
                p = float(P_SCHED[i])
                if i >= DIRECT_ITERS:
                    # rank rr = p - khi = ssum/2 + (p-1024), clamped <= 8
                    nc.vector.tensor_scalar(rr[g][:], ssum[g][:], 0.5,
                                            p - 1024.0, A.mult, A.add)
                    nc.vector.tensor_scalar(rr[g][:], rr[g][:], 8.0, None,
                                            A.min)
                    # masked top8: vals = (au <= hi) * au
                    nc.vector.scalar_tensor_tensor(
                        u[g][:], au[g][:], hi[g][:], au[g][:],
                        A.is_le, A.mult)
                    nc.vector.max(top8[g][:], u[g][:])
                    nc.vector.tensor_scalar_add(rr5[g][:], rr[g][:], 0.5)
                    nc.vector.scalar_tensor_tensor(
                        t8[g][:], io8[:], rr[g][:], top8[g][:],
                        A.is_ge, A.mult)
                    nc.vector.scalar_tensor_tensor(
                        t8[g][:], io8[:], rr5[g][:], t8[g][:],
                        A.is_le, A.mult)
                    nc.vector.tensor_reduce(Tsc[g][:], t8[g][:], AX.X, A.add)
                    # fallback: rr <= 0.25 (khi >= p) -> Tsc = hi
                    nc.vector.tensor_scalar(m0[g][:], rr[g][:], 0.25, None,
                                            A.is_le)
                    nc.vector.tensor_tensor(fb[g][:], m0[g][:], hi[g][:],
                                            A.mult)
                    nc.vector.tensor_tensor(Tsc[g][:], Tsc[g][:], fb[g][:],
                                            A.add)

            def emit_shrink(g, i):
                # ---- shrink + overshoot ----
                # nq = (au <= Tsc)*q; d = e - nq; w = (|d|+eps)/a;
                # r = 1/w; xn = x + (1+r)*d
                nc.vector.scalar_tensor_tensor(u[g][:], au[g][:], Tsc[g][:],
                                               q[g][:], A.is_le, A.mult)
                # chunked tail: chunk 0 completes early so the next gain
                # (Abs/Exp on ACT) starts while chunk 1 is still on DVE
                for c in range(NCHUNK):
                    nc.vector.tensor_tensor(cs(q[g], c), cs(e[g], c),
                                            cs(u[g], c), A.subtract)
                    nc.vector.scalar_tensor_tensor(
                        cs(au[g], c), cs(q[g], c), -1.0, cs(q[g], c),
                        A.mult, A.max)
                    nc.vector.tensor_scalar(cs(au[g], c), cs(au[g], c),
                                            float(1.0 / aa_[i]),
                                            float(EPS / aa_[i]),
                                            A.mult, A.add)
                    nc.vector.reciprocal_approx_fast(cs(au[g], c),
                                                     cs(au[g], c))
                    nc.vector.scalar_tensor_tensor(
                        cs(au[g], c), cs(au[g], c), 1.0, cs(q[g], c),
                        A.add, A.mult)
                    nc.vector.tensor_tensor(cs(x[g], c), cs(x[g], c),
                                            cs(au[g], c), A.add)

            # ---- software-pipelined emission ----
            # topk(g) is emitted before the other group's mmB so the ACT
            # bisect chain overlaps the PE matmul block (in-order queues:
            # emission order ~ execution order per engine).
            emit_mmA(0, 0)
            emit_mmB(0, 0)
            for i in range(n_iters):
                emit_mmA(1, i)
                emit_topk(0, i)
                emit_rank(0, i)
                emit_mmB(1, i)
                emit_shrink(0, i)
                emit_topk(1, i)
                emit_rank(1, i)
                if i + 1 < n_iters:
                    emit_mmA(0, i + 1)
                emit_shrink(1, i)
                if i + 1 < n_iters:
                    emit_mmB(0, i + 1)

            for g in range(G):
                nc.sync.dma_start(out_d[128 * g:128 * (g + 1), :], x[g][:])

    nc.finalize()
    return nc


def _prep_inputs(y, phi, W):
    import ml_dtypes
    bf16 = ml_dtypes.bfloat16

    def mask13(v):
        return (np.ascontiguousarray(v).view(np.uint32)
                & np.uint32(0xFFFFE000)).view(np.float32)

    phiT = np.ascontiguousarray(
        phi.T.reshape(16, 128, M).transpose(1, 0, 2)).astype(np.float32)
    phh = phiT.astype(bf16)
    phl = (phiT - phh.astype(np.float32)).astype(bf16)
    Wm = np.ascontiguousarray(
        W.reshape(4, 128, N).transpose(1, 0, 2)).astype(np.float32)
    wh_ = mask13(Wm)
    wl_ = mask13((Wm - wh_).astype(np.float32))
    yT_full = np.ascontiguousarray(y.T)  # (M, B)
    in_maps = []
    for c in range(NCORES):
        yTc = yT_full[:, c * BL:(c + 1) * BL]
        yTs = np.ascontiguousarray(
            yTc.reshape(4, 128, BL).transpose(1, 0, 2)).astype(np.float32)
        in_maps.append({"phh": phh, "phl": phl, "wh": wh_, "wl": wl_,
                        "yT": yTs})
    return in_maps


def kernel(y, phi, W, gamma, theta, a, v, vu, theta_initial, _profile=None):
    from concourse.bass_utils import run_bass_kernel_spmd

    import os
    scal = (tuple(np.asarray(gamma, np.float64).tolist()),
            tuple(np.asarray(theta, np.float64).tolist()),
            tuple(np.asarray(a, np.float64).tolist()),
            tuple(np.asarray(v, np.float64).tolist()),
            tuple(np.asarray(vu, np.float64).tolist()),
            float(theta_initial))
    n_iters = int(os.environ.get("KERNEL_ITERS", K))
    key = (scal, n_iters)
    if _CACHE.get("key") != key:
        _CACHE["nc"] = _build(scal, n_iters=n_iters)
        _CACHE["key"] = key
    nc = _CACHE["nc"]
    in_maps = _prep_inputs(np.asarray(y, np.float32),
                           np.asarray(phi, np.float32),
                           np.asarray(W, np.float32))
    kw = dict(_profile) if _profile else {}
    res = run_bass_kernel_spmd(nc, in_maps, list(range(NCORES)), **kw)
    out = np.empty((B, N), np.float32)
    for c in range(NCORES):
        out[c * BL:(c + 1) * BL, :] = res.results[c]["out"]
    if _profile:
        _CACHE["last_results"] = res
    return out
